# revision 1
# baseline (speedup 1.0000x reference)
"""Trainium2 Bass kernel for batch-attention block (B=64, C=256, L=4096).

Sequence-parallel over L across 8 cores (Lc=512 per core). Math:
  g = (WkT@Wq/sqrt(C))^T-conv of x ; attT[d,b,l] = sum_c g[c,d,l] x[c,b,l]
  e = exp(attT) (no max-subtract; values are O(+-8))
  s[b,l] = sum_d e[d,b,l] ; softmax normalization deferred: rs = 1/s folded
  into the PSUM evacuation of the mix matmul.
  vB[d,c,l] = (Wv x) computed directly in batch-major layout on PE.
  virt[c,b,l] = (sum_d vB[d,c,l] e[d,b,l]) * rs[b,l]   (kept in SBUF, bf16)
  GroupNorm stats per sample b over (C, Lc) via bn_stats, AllReduce'd
  across cores (sum of per-(c,b) mean and E[x^2] planes), then
  rc = relu(virt * A_b + B_b), y = x + Wc-conv(rc), out = Wout-conv(y).
"""

import numpy as np
import ml_dtypes
from contextlib import ExitStack

from concourse import bass, bacc, tile, mybir
from concourse.bass_utils import run_bass_kernel_spmd

F32 = mybir.dt.float32
BF16 = mybir.dt.bfloat16
AF = mybir.ActivationFunctionType
BF16NP = ml_dtypes.bfloat16

NCORES = 8
B = 64
C = 256
L = 4096
LC = L // NCORES          # 512 positions per core
LB = 32                   # positions per block
NBLK = LC // LB           # 16
NPAIR = LB // 2           # 16 pairs per block
NGRP = 4                  # pairs per att-psum group
EPS = 1e-5

_CACHE = {}


def build(nblk=NBLK, do_stats=True, do_coll=True, do_s2=True):
    nc = bacc.Bacc("TRN2", target_bir_lowering=False, debug=False,
                   num_devices=NCORES)

    xb = nc.dram_tensor("xb", [2, 128, LC, B], BF16, kind="ExternalInput")
    xf = nc.dram_tensor("xf", [B, 2, 128, LC], F32, kind="ExternalInput")
    wg = nc.dram_tensor("wg", [2, 128, C], BF16, kind="ExternalInput")
    wv = nc.dram_tensor("wv", [2, 128, C], BF16, kind="ExternalInput")
    wc = nc.dram_tensor("wc", [2, 128, C], BF16, kind="ExternalInput")
    wo = nc.dram_tensor("wo", [2, 128, C], BF16, kind="ExternalInput")
    gb = nc.dram_tensor("gb", [2, 2, 128], F32, kind="ExternalInput")
    out = nc.dram_tensor("out", [B, 2, 128, LC], F32, kind="ExternalOutput")

    with tile.TileContext(nc) as tc, ExitStack() as top:
        persist = top.enter_context(tc.tile_pool(name="persist", bufs=1))
        dram = top.enter_context(tc.tile_pool(name="dram", bufs=1, space="DRAM"))

        # ---- persistent SBUF: weights, virt, constants -------------------
        wg_sb, wv_sb, wc_sb, wo_sb = [], [], [], []
        for nm, dr, lst in (("wg", wg, wg_sb), ("wv", wv, wv_sb),
                            ("wc", wc, wc_sb), ("wo", wo, wo_sb)):
            for ct in range(2):
                t = persist.tile([128, C], BF16, tag=f"{nm}{ct}")
                nc.sync.dma_start(out=t[:], in_=dr.ap()[ct])
                lst.append(t)

        gam_sb, bet_sb = [], []
        for ct in range(2):
            t = persist.tile([128, 1], F32, tag=f"gam{ct}")
            nc.sync.dma_start(
                out=t[:], in_=gb.ap()[0, ct].rearrange("(p one) -> p one", one=1))
            gam_sb.append(t)
            t = persist.tile([128, 1], F32, tag=f"bet{ct}")
            nc.sync.dma_start(
                out=t[:], in_=gb.ap()[1, ct].rearrange("(p one) -> p one", one=1))
            bet_sb.append(t)

        ones2 = persist.tile([128, 2], BF16, tag="ones2")
        nc.vector.memset(ones2[:], 0.0)
        nc.vector.memset(ones2[0:64, 0:1], 1.0)
        nc.vector.memset(ones2[64:128, 1:2], 1.0)
        ones1 = persist.tile([128, 1], F32, tag="ones1")
        nc.vector.memset(ones1[:], 1.0)

        virt = [persist.tile([128, B * LC], BF16, tag=f"virt{ct}", name=f"virt{ct}")
                for ct in range(2)]
        stt2 = [persist.tile([128, B, 2, 6], F32, tag=f"stt{ct}", name=f"stt{ct}")
                for ct in range(2)]

        rs_dr = dram.tile([NBLK, 2, NPAIR * 128], BF16)

        # =================== STAGE 1 =====================================
        with ExitStack() as s1:
            sp = s1.enter_context(tc.tile_pool(name="s1sb", bufs=2))
            sp1 = s1.enter_context(tc.tile_pool(name="s1sb1", bufs=1))
            pw = s1.enter_context(tc.tile_pool(name="pw", bufs=3, space="PSUM"))
            pm = s1.enter_context(tc.tile_pool(name="pm", bufs=2, space="PSUM"))

            for blk in range(nblk):
                # ---- load x block (bf16, c-major, cols = (l, b)) --------
                xbf = [sp.tile([128, LB * B], BF16, tag=f"xbf{ct}", name=f"xbf{ct}")
                       for ct in range(2)]
                for ct in range(2):
                    nc.sync.dma_start(
                        out=xbf[ct][:],
                        in_=xb.ap()[ct, :, blk * LB:(blk + 1) * LB, :])

                # ---- g conv: g = lhsT_g.T @ x ---------------------------
                g_sb = [sp1.tile([128, LB * B], BF16, tag=f"g{ct}", name=f"g{ct}")
                        for ct in range(2)]
                for ct1 in range(2):
                    for ch in range(2):
                        gp = pw.tile([128, 1024], F32, tag="w", name="gp")
                        for h in range(2):
                            for ct2 in range(2):
                                nc.tensor.matmul(
                                    gp[:, h * 512:(h + 1) * 512],
                                    wg_sb[ct2][:, ct1 * 128:(ct1 + 1) * 128],
                                    xbf[ct2][:, ch * 1024 + h * 512:
                                              ch * 1024 + (h + 1) * 512],
                                    start=(ct2 == 0), stop=(ct2 == 1))
                        nc.scalar.copy(g_sb[ct1][:, ch * 1024:(ch + 1) * 1024],
                                       gp[:])

                # ---- att (paired, junk halves) + exp + sums -------------
                e_sb = sp.tile([128, NPAIR * 128], BF16, tag="e")
                rs_sb = sp.tile([2, NPAIR * 128], BF16, tag="rs")
                for grp in range(2):
                    ap_ = pw.tile([128, 1024], F32, tag="w", name="attp")
                    for pi in range(8):
                        p = grp * 8 + pi
                        for kt in range(2):
                            nc.tensor.matmul(
                                ap_[:, pi * 128:(pi + 1) * 128],
                                g_sb[kt][:, p * 128:(p + 1) * 128],
                                xbf[kt][:, p * 128:(p + 1) * 128],
                                start=(kt == 0), stop=(kt == 1))
                    nc.scalar.activation(
                        e_sb[:, grp * 1024:(grp + 1) * 1024], ap_[:], AF.Exp)
                    s_ps = pw.tile([2, 1024], F32, tag="w", name="sp")
                    for h in range(2):
                        nc.tensor.matmul(
                            s_ps[:, h * 512:(h + 1) * 512], ones2[:],
                            e_sb[:, grp * 1024 + h * 512:
                                 grp * 1024 + (h + 1) * 512],
                            start=True, stop=True)
                    with nc.allow_low_precision(reason="softmax rs in bf16"):
                        nc.vector.reciprocal(
                            rs_sb[:, grp * 1024:(grp + 1) * 1024], s_ps[:])

                # rs -> DRAM -> partition-broadcast tiles (per parity)
                nc.sync.dma_start(out=rs_dr[blk], in_=rs_sb[:])
                rs_bc = [sp.tile([128, NPAIR * B], BF16, tag=f"rsbc{par}", name=f"rsbc{par}")
                         for par in range(2)]
                for par in range(2):
                    src = bass.AP(
                        tensor=rs_dr.tensor,
                        offset=rs_dr.offset + blk * 2 * NPAIR * 128
                        + par * NPAIR * 128 + par * 64,
                        ap=[[0, 128], [128, NPAIR], [1, B]])
                    nc.sync.dma_start(out=rs_bc[par][:], in_=src)

                # ---- vB: per pair one (128,128,256) matmul --------------
                vb_sb = sp.tile([128, NPAIR * C], BF16, tag="vb")
                for ph in range(NPAIR // 4):
                    vp = pw.tile([128, 1024], F32, tag="w", name="vbp")
                    for pi in range(4):
                        p = ph * 4 + pi
                        for kt in range(2):
                            nc.tensor.matmul(
                                vp[:, pi * 256:(pi + 1) * 256],
                                xbf[kt][:, p * 128:(p + 1) * 128],
                                wv_sb[kt][:],
                                start=(kt == 0), stop=(kt == 1))
                    nc.scalar.copy(vb_sb[:, ph * 1024:(ph + 1) * 1024], vp[:])

                # ---- mix (row-tiled T0/T8) + fused rs-scaled evac -------
                for ct in range(2):
                    for par in range(2):
                        for ch in range(2):
                            mp = pm.tile([128, 512], F32, tag="mixp")
                            for i in range(8):
                                p = ch * 8 + i
                                r0 = par * 64
                                nc.tensor.matmul(
                                    mp[:, i * 64:(i + 1) * 64],
                                    vb_sb[r0:r0 + 64,
                                          p * 256 + ct * 128:
                                          p * 256 + (ct + 1) * 128],
                                    e_sb[r0:r0 + 64,
                                         p * 128 + par * 64:
                                         p * 128 + par * 64 + 64],
                                    start=True, stop=True,
                                    tile_position=(r0, 0))
                            virt4 = virt[ct].rearrange(
                                "p (b hl two) -> p b hl two", b=B, two=2)
                            hl0 = blk * 16 + ch * 8
                            dst = virt4[:, :, hl0:hl0 + 8, par]
                            nc.vector.tensor_tensor(
                                out=dst, in0=mp[:].rearrange(
                                    "p (i b) -> p b i", i=8),
                                in1=rs_bc[par][:, ch * 512:(ch + 1) * 512]
                                .rearrange("p (i b) -> p b i", i=8),
                                op=mybir.AluOpType.mult)

                # ---- incremental GroupNorm stats (hide behind stage 1) --
                if do_stats and blk in (nblk // 2 - 1, nblk - 1):
                    h = 0 if blk == nblk // 2 - 1 else 1
                    for ct in range(2):
                        for b in range(B):
                            nc.vector.bn_stats(
                                out=stt2[ct][:, b, h, :],
                                in_=virt[ct][:, b * LC + h * (LC // 2):
                                             b * LC + (h + 1) * (LC // 2)])

        # =================== STAGE 1.5: stats + collective ===============
        with ExitStack() as s15:
          if do_stats:
              st = s15.enter_context(tc.tile_pool(name="stsb", bufs=1))  # noqa
              stp = s15.enter_context(tc.tile_pool(name="stps", bufs=1,
                                                   space="PSUM"))
              packs = []
              for ct in range(2):
                  mv = st.tile([128, B, 2], F32, tag=f"mv{ct}", name=f"mv{ct}")
                  for b in range(B):
                      nc.vector.bn_aggr(out=mv[:, b, :],
                                        in_=stt2[ct][:, b, :, :])
                  pack = st.tile([128, 2 * B], F32, tag=f"pack{ct}")
                  # cols 0..63 = mean, 64..127 = E[x^2] = var + mean^2
                  nc.vector.tensor_copy(pack[:, 0:B], mv[:, :, 0])
                  nc.vector.tensor_tensor(out=pack[:, B:2 * B], in0=mv[:, :, 0],
                                          in1=mv[:, :, 0],
                                          op=mybir.AluOpType.mult)
                  nc.vector.tensor_tensor(out=pack[:, B:2 * B],
                                          in0=pack[:, B:2 * B], in1=mv[:, :, 1],
                                          op=mybir.AluOpType.add)
                  packs.append(pack)

              sum_ps = stp.tile([1, 2 * B], F32, tag="sump")
              for ct in range(2):
                  nc.tensor.matmul(sum_ps[:], ones1[:], packs[ct][:],
                                   start=(ct == 0), stop=(ct == 1))
              loc = st.tile([1, 2 * B], F32, tag="loc")
              nc.vector.tensor_copy(loc[:], sum_ps[:])

              cc_in = dram.tile([1, 2 * B], F32)
              cc_out = dram.tile([1, 2 * B], F32)
              nc.sync.dma_start(out=cc_in[:], in_=loc[:])
              if do_coll:
                  nc.gpsimd.collective_compute(
                      "AllReduce", mybir.AluOpType.add,
                      replica_groups=[list(range(NCORES))],
                      ins=[cc_in.opt()], outs=[cc_out.opt()])
                  bc_src = cc_out
              else:
                  bc_src = cc_in
              stat_bc = st.tile([128, 2 * B], F32, tag="statbc")
              nc.sync.dma_start(
                  out=stat_bc[:],
                  in_=bass.AP(tensor=bc_src.tensor, offset=bc_src.offset,
                              ap=[[0, 128], [1, 2 * B]]))

              # global mean/var/rstd  (sum over 2048 cells of (c, core))
              inv = 1.0 / (C * NCORES)
              mean_g = persist.tile([128, B], F32, tag="meang")
              nc.scalar.mul(mean_g[:], stat_bc[:, 0:B], inv)
              var_g = st.tile([128, B], F32, tag="varg")
              nc.scalar.mul(var_g[:], stat_bc[:, B:2 * B], inv)
              msq = st.tile([128, B], F32, tag="msq")
              nc.vector.tensor_tensor(out=msq[:], in0=mean_g[:], in1=mean_g[:],
                                      op=mybir.AluOpType.mult)
              nc.vector.tensor_tensor(out=var_g[:], in0=var_g[:], in1=msq[:],
                                      op=mybir.AluOpType.subtract)
              sd = st.tile([128, B], F32, tag="sd")
              eps_t = st.tile([128, 1], F32, tag="epst")
              nc.vector.memset(eps_t[:], EPS)
              nc.scalar.activation(sd[:], var_g[:], AF.Sqrt, bias=eps_t[:])
              rstd = st.tile([128, B], F32, tag="rstd")
              nc.vector.reciprocal(rstd[:], sd[:])

              # A[ct] = rstd * gamma_c ; Bb[ct] = beta_c - mean * A
              A_sb = [persist.tile([128, B], F32, tag=f"A{ct}", name=f"A{ct}") for ct in range(2)]
              B_sb = [persist.tile([128, B], F32, tag=f"Bb{ct}", name=f"Bb{ct}") for ct in range(2)]
              for ct in range(2):
                  nc.vector.tensor_scalar_mul(A_sb[ct][:], rstd[:], gam_sb[ct][:])
                  tmp = st.tile([128, B], F32, tag="tmpB")
                  nc.vector.tensor_tensor(out=tmp[:], in0=mean_g[:],
                                          in1=A_sb[ct][:],
                                          op=mybir.AluOpType.mult)
                  nc.scalar.activation(B_sb[ct][:], tmp[:], AF.Identity,
                                       bias=bet_sb[ct][:], scale=-1.0)

        # =================== STAGE 2 =====================================
        with ExitStack() as s2:
          if do_s2:
              p2 = s2.enter_context(tc.tile_pool(name="s2sb", bufs=3))
              pc = s2.enter_context(tc.tile_pool(name="pc", bufs=4, space="PSUM"))
              po = s2.enter_context(tc.tile_pool(name="po", bufs=4, space="PSUM"))

              for b in range(B):
                  rc = [p2.tile([128, LC], BF16, tag=f"rc{ct}", name=f"rc{ct}") for ct in range(2)]
                  for ct in range(2):
                      nc.scalar.activation(
                          rc[ct][:], virt[ct][:, b * LC:(b + 1) * LC], AF.Relu,
                          bias=B_sb[ct][:, b:b + 1], scale=A_sb[ct][:, b:b + 1])
                  xf_t = [p2.tile([128, LC], F32, tag=f"xt{ct}", name=f"xt{ct}") for ct in range(2)]
                  for ct in range(2):
                      nc.sync.dma_start(out=xf_t[ct][:], in_=xf.ap()[b, ct])
                  y = [p2.tile([128, LC], BF16, tag=f"y{ct}", name=f"y{ct}") for ct in range(2)]
                  for ot in range(2):
                      cp = pc.tile([128, LC], F32, tag="wcp")
                      for kt in range(2):
                          nc.tensor.matmul(cp[:],
                                           wc_sb[kt][:, ot * 128:(ot + 1) * 128],
                                           rc[kt][:],
                                           start=(kt == 0), stop=(kt == 1))
                      nc.vector.tensor_tensor(out=y[ot][:], in0=cp[:],
                                              in1=xf_t[ot][:],
                                              op=mybir.AluOpType.add)
                  for ot in range(2):
                      op_ = po.tile([128, LC], F32, tag="wop")
                      for kt in range(2):
                          nc.tensor.matmul(op_[:],
                                           wo_sb[kt][:, ot * 128:(ot + 1) * 128],
                                           y[kt][:],
                                           start=(kt == 0), stop=(kt == 1))
                      o_sb = p2.tile([128, LC], F32, tag=f"o{ot}")
                      nc.scalar.copy(o_sb[:], op_[:])
                      nc.sync.dma_start(out=out.ap()[b, ot], in_=o_sb[:])

    nc.compile()
    return nc


def kernel(x, Wq, Wk, Wv, Wc, Wout, gamma, beta):
    x = np.asarray(x)
    Wq, Wk, Wv, Wc, Wout = map(np.asarray, (Wq, Wk, Wv, Wc, Wout))
    gamma, beta = np.asarray(gamma), np.asarray(beta)

    if "nc" not in _CACHE:
        _CACHE["nc"] = build()
    nc = _CACHE["nc"]

    # host weight prep
    M = (Wq.T @ Wk) / np.sqrt(np.float32(C))
    wg = np.ascontiguousarray(M.T.reshape(2, 128, C)).astype(BF16NP)
    wv = np.ascontiguousarray(Wv.T.reshape(2, 128, C)).astype(BF16NP)
    wc = np.ascontiguousarray(Wc.T.reshape(2, 128, C)).astype(BF16NP)
    wo = np.ascontiguousarray(Wout.T.reshape(2, 128, C)).astype(BF16NP)
    gb = np.stack([gamma.reshape(2, 128), beta.reshape(2, 128)]).astype(np.float32)
    gb = np.ascontiguousarray(gb)

    in_maps = []
    for m in range(NCORES):
        xs = np.ascontiguousarray(x[:, :, m * LC:(m + 1) * LC], dtype=np.float32)
        xb = np.ascontiguousarray(
            xs.transpose(1, 2, 0).reshape(2, 128, LC, B)).astype(BF16NP)
        xf = np.ascontiguousarray(xs.reshape(B, 2, 128, LC))
        in_maps.append(dict(xb=xb, xf=xf, wg=wg, wv=wv, wc=wc, wo=wo, gb=gb))

    import os
    trace = bool(int(os.environ.get("KERNEL_TRACE", "0")))
    res = run_bass_kernel_spmd(nc, in_maps, core_ids=list(range(NCORES)),
                               trace=trace)
    _CACHE["last_result"] = res
    shards = [res.results[m]["out"].reshape(B, C, LC) for m in range(NCORES)]
    return np.concatenate(shards, axis=2)



# revision 31
# speedup vs baseline: 1.2783x; 1.2783x over previous
"""Trainium2 Bass kernel for batch-attention block (B=64, C=256, L=4096).

Sequence-parallel over L across 8 cores (Lc=512 per core). Math:
  g = (WkT@Wq/sqrt(C))^T-conv of x ; attT[d,b,l] = sum_c g[c,d,l] x[c,b,l]
  e = exp(attT) (no max-subtract; values are O(+-8))
  s[b,l] = sum_d e[d,b,l] ; softmax normalization deferred: rs = 1/s folded
  into the PSUM evacuation of the mix matmul.
  vB[d,c,l] = (Wv x) computed directly in batch-major layout on PE.
  virt[c,b,l] = (sum_d vB[d,c,l] e[d,b,l]) * rs[b,l]   (kept in SBUF, bf16)
  rs is computed on a [128,16]-regrouped copy of s (128 DVE lanes for the
  8-cyc/elem divide), and applied via scalar PSUM->bf16 copy + DVE 2x mult.
  virt cols are (blk, ch, par, i, b); the host un-permutes output columns.
  GroupNorm stats per sample b over (C, Lc) via bn_stats, AllReduce'd
  across cores (sum of per-(c,b) mean and E[x^2] planes), then
  rc = relu(virt * A_b + B_b), y = x + Wc-conv(rc), out = Wout-conv(y).
"""

import numpy as np
import ml_dtypes
from contextlib import ExitStack

from concourse import bass, bacc, tile, mybir
from concourse.bass_utils import run_bass_kernel_spmd

F32 = mybir.dt.float32
BF16 = mybir.dt.bfloat16
AF = mybir.ActivationFunctionType
BF16NP = ml_dtypes.bfloat16

NCORES = 8
B = 64
C = 256
L = 4096
LC = L // NCORES          # 512 positions per core
LB = 32                   # positions per block
NBLK = LC // LB           # 16
NPAIR = LB // 2           # 16 pairs per block
NGRP = 4                  # pairs per att-psum group
EPS = 1e-5

_CACHE = {}


# virt/out column order is (blk, ch, i, par, b) which equals natural
# l-order: l = ch*16 + i*2 + par for z = (ch, i, par).
PERM = np.arange(LC)


def build(nblk=NBLK, do_stats=True, do_coll=True, do_s2=True, dbg=False):
    nc = bacc.Bacc("TRN2", target_bir_lowering=False, debug=False,
                   num_devices=NCORES)
    if dbg:
        dbg_s = nc.dram_tensor("dbg_s", [2, 2048], BF16, kind="ExternalOutput")
        dbg_s128 = nc.dram_tensor("dbg_s128", [128, 16], BF16,
                                  kind="ExternalOutput")
        dbg_rs128 = nc.dram_tensor("dbg_rs128", [128, 16], BF16,
                                   kind="ExternalOutput")
        dbg_rsbc = nc.dram_tensor("dbg_rsbc", [128, 2048], BF16,
                                  kind="ExternalOutput")
        dbg_e = nc.dram_tensor("dbg_e", [128, 2048], BF16,
                               kind="ExternalOutput")
        dbg_virt = nc.dram_tensor("dbg_virt", [2, 128, 2048], BF16,
                                  kind="ExternalOutput")

    xb = nc.dram_tensor("xb", [2, 128, LC, B], BF16, kind="ExternalInput")
    xf = nc.dram_tensor("xf", [B, 2, 128, LC], BF16, kind="ExternalInput")
    wg = nc.dram_tensor("wg", [2, 128, C], BF16, kind="ExternalInput")
    wv = nc.dram_tensor("wv", [2, 128, C], BF16, kind="ExternalInput")
    wc = nc.dram_tensor("wc", [2, 128, C], BF16, kind="ExternalInput")
    wo = nc.dram_tensor("wo", [2, 128, C], BF16, kind="ExternalInput")
    gb = nc.dram_tensor("gb", [2, 2, 128], F32, kind="ExternalInput")
    out = nc.dram_tensor("out", [B, 2, 128, LC], BF16, kind="ExternalOutput")

    with tile.TileContext(nc) as tc, ExitStack() as top:
        persist = top.enter_context(tc.tile_pool(name="persist", bufs=1))
        dram = top.enter_context(tc.tile_pool(name="dram", bufs=1, space="DRAM"))

        # ---- persistent SBUF: weights, virt, constants -------------------
        wg_sb, wv_sb, wc_sb, wo_sb = [], [], [], []
        for nm, dr, lst in (("wg", wg, wg_sb), ("wv", wv, wv_sb),
                            ("wc", wc, wc_sb), ("wo", wo, wo_sb)):
            for ct in range(2):
                t = persist.tile([128, C], BF16, tag=f"{nm}{ct}")
                nc.sync.dma_start(out=t[:], in_=dr.ap()[ct])
                lst.append(t)

        gam_sb, bet_sb = [], []
        for ct in range(2):
            t = persist.tile([128, 1], F32, tag=f"gam{ct}")
            nc.sync.dma_start(
                out=t[:], in_=gb.ap()[0, ct].rearrange("(p one) -> p one", one=1))
            gam_sb.append(t)
            t = persist.tile([128, 1], F32, tag=f"bet{ct}")
            nc.sync.dma_start(
                out=t[:], in_=gb.ap()[1, ct].rearrange("(p one) -> p one", one=1))
            bet_sb.append(t)

        ones2 = persist.tile([128, 2], BF16, tag="ones2")
        nc.vector.memset(ones2[:], 0.0)
        nc.vector.memset(ones2[0:64, 0:1], 1.0)
        nc.vector.memset(ones2[64:128, 1:2], 1.0)
        ones1 = persist.tile([128, 1], F32, tag="ones1")
        nc.vector.memset(ones1[:], 1.0)

        # rs broadcast tiles with zeroed junk quadrants (rows par != col
        # par); zeros persist because the DMAs only write matched slots.
        rsJ = [persist.tile([128, NPAIR * 128], BF16, tag=f"rsJ{k}",
                            name=f"rsJ{k}")
               for k in range(2)]
        for k in range(2):
            nc.vector.memset(rsJ[k][:], 0.0)

        virt = [persist.tile([128, B * LC], BF16, tag=f"virt{ct}", name=f"virt{ct}")
                for ct in range(2)]
        stt2 = [persist.tile([128, B, 2, 6], F32, tag=f"stt{ct}", name=f"stt{ct}")
                for ct in range(2)]

        # softmax-denominator plumbing: s [2, 2048] per block -> regrouped
        # [128, 16] so the 8-cyc/elem divide uses all 128 DVE lanes.
        s_dr = dram.tile([NBLK, 2, NPAIR * 128], BF16)
        rs2_dr = dram.tile([NBLK, 128, NPAIR], BF16)

        # =================== STAGE 1 =====================================
        with ExitStack() as s1:
            sp = s1.enter_context(tc.tile_pool(name="s1sb", bufs=2))
            sp1 = s1.enter_context(tc.tile_pool(name="s1sb1", bufs=1))
            pw = s1.enter_context(tc.tile_pool(name="pw", bufs=3, space="PSUM"))
            pm = s1.enter_context(tc.tile_pool(name="pm", bufs=2, space="PSUM"))

            for blk in range(nblk):
                # ---- load x block (bf16, c-major, cols = (l, b)) --------
                xbf = [sp.tile([128, LB * B], BF16, tag=f"xbf{ct}", name=f"xbf{ct}")
                       for ct in range(2)]
                for ct in range(2):
                    nc.sync.dma_start(
                        out=xbf[ct][:],
                        in_=xb.ap()[ct, :, blk * LB:(blk + 1) * LB, :])

                # ---- g conv: g = lhsT_g.T @ x ---------------------------
                g_sb = [sp1.tile([128, LB * B], BF16, tag=f"g{ct}", name=f"g{ct}")
                        for ct in range(2)]
                for ct1 in range(2):
                    for ch in range(2):
                        gp = pw.tile([128, 1024], F32, tag="w", name="gp")
                        for h in range(2):
                            for ct2 in range(2):
                                nc.tensor.matmul(
                                    gp[:, h * 512:(h + 1) * 512],
                                    wg_sb[ct2][:, ct1 * 128:(ct1 + 1) * 128],
                                    xbf[ct2][:, ch * 1024 + h * 512:
                                              ch * 1024 + (h + 1) * 512],
                                    start=(ct2 == 0), stop=(ct2 == 1))
                        nc.scalar.copy(g_sb[ct1][:, ch * 1024:(ch + 1) * 1024],
                                       gp[:])

                # ---- att (paired, junk halves) + exp + sums -------------
                e_sb = sp.tile([128, NPAIR * 128], BF16, tag="e")
                for grp in range(2):
                    ap_ = pw.tile([128, 1024], F32, tag="w", name="attp")
                    for pi in range(8):
                        p = grp * 8 + pi
                        for kt in range(2):
                            nc.tensor.matmul(
                                ap_[:, pi * 128:(pi + 1) * 128],
                                g_sb[kt][:, p * 128:(p + 1) * 128],
                                xbf[kt][:, p * 128:(p + 1) * 128],
                                start=(kt == 0), stop=(kt == 1))
                    nc.scalar.activation(
                        e_sb[:, grp * 1024:(grp + 1) * 1024], ap_[:], AF.Exp)
                    s_ps = pw.tile([2, 1024], F32, tag="w", name="sp")
                    for h in range(2):
                        nc.tensor.matmul(
                            s_ps[:, h * 512:(h + 1) * 512], ones2[:],
                            e_sb[:, grp * 1024 + h * 512:
                                 grp * 1024 + (h + 1) * 512],
                            start=True, stop=True)
                    # s -> SBUF bf16 -> DRAM; divide happens after the
                    # [128, 16] regroup so it uses all DVE lanes.
                    s_sb = sp.tile([2, 1024], BF16, tag="ssb")
                    nc.scalar.copy(s_sb[:], s_ps[:])
                    nc.sync.dma_start(
                        out=s_dr[blk][:, grp * 1024:(grp + 1) * 1024],
                        in_=s_sb[:])

                # s -> [128, 16] (part = par*64 + pair*4 + b//16, col =
                # b%16), 1/s, -> DRAM laid out [par, pair, b] so the
                # broadcast reads are inner-contiguous.
                s128 = sp.tile([128, NPAIR], BF16, tag="s128")
                for par in range(2):
                    nc.sync.dma_start(
                        out=s128[par * B:(par + 1) * B, :],
                        in_=bass.AP(tensor=s_dr.tensor,
                                    offset=s_dr.offset + blk * 2 * 2048
                                    + par * 2112,
                                    ap=[[128, NPAIR], [16, 4], [1, 16]]))
                rs128 = sp.tile([128, NPAIR], BF16, tag="rs128")
                with nc.allow_low_precision(reason="softmax rs in bf16"):
                    nc.vector.reciprocal(rs128[:], s128[:])
                nc.sync.dma_start(out=rs2_dr[blk], in_=rs128[:])
                rs_bc = rsJ[blk % 2]
                rsv = rs_bc.rearrange("p (pr two b) -> p pr two b",
                                      two=2, b=B)
                for rp in range(2):
                    src = bass.AP(
                        tensor=rs2_dr.tensor,
                        offset=rs2_dr.offset + blk * 128 * NPAIR
                        + rp * NPAIR * B,
                        ap=[[0, 64], [B, NPAIR], [1, B]])
                    nc.sync.dma_start(
                        out=rsv[rp * 64:(rp + 1) * 64, :, rp, :], in_=src)
                # e_n = e * rs (junk quadrants multiplied by 0)
                e_n = sp.tile([128, NPAIR * 128], BF16, tag="en")
                nc.vector.tensor_tensor(out=e_n[:], in0=e_sb[:],
                                        in1=rs_bc[:],
                                        op=mybir.AluOpType.mult)
                if dbg and blk == 0:
                    nc.sync.dma_start(out=dbg_s.ap(), in_=s_dr[0])
                    nc.sync.dma_start(out=dbg_s128.ap(), in_=s128[:])
                    nc.sync.dma_start(out=dbg_rs128.ap(), in_=rs128[:])
                    nc.sync.dma_start(out=dbg_rsbc.ap(), in_=e_n[:])
                    nc.sync.dma_start(out=dbg_e.ap(), in_=e_sb[:])

                # ---- vB: per pair one (128,128,256) matmul --------------
                vb_sb = sp.tile([128, NPAIR * C], BF16, tag="vb")
                for ph in range(NPAIR // 4):
                    vp = pw.tile([128, 1024], F32, tag="w", name="vbp")
                    for pi in range(4):
                        p = ph * 4 + pi
                        for kt in range(2):
                            nc.tensor.matmul(
                                vp[:, pi * 256:(pi + 1) * 256],
                                xbf[kt][:, p * 128:(p + 1) * 128],
                                wv_sb[kt][:],
                                start=(kt == 0), stop=(kt == 1))
                    nc.vector.tensor_copy(vb_sb[:, ph * 1024:(ph + 1) * 1024],
                                          vp[:])

                # ---- mix: one [128,128]x[128,128] MM per (pair, ct) -----
                # e_n junk quadrants are zero, so a full-width MM gives
                # both parities; virt cols = (blk, ch, i, par, b) =
                # natural l-order (host permutation is identity).
                for ct in range(2):
                    for ch in range(2):
                        for h2 in range(2):
                            mp = pm.tile([128, 512], F32, tag="mixp")
                            for i4 in range(4):
                                i = h2 * 4 + i4
                                p = ch * 8 + i
                                nc.tensor.matmul(
                                    mp[:, i4 * 128:(i4 + 1) * 128],
                                    vb_sb[:, p * 256 + ct * 128:
                                          p * 256 + (ct + 1) * 128],
                                    e_n[:, p * 128:(p + 1) * 128],
                                    start=True, stop=True)
                            c0 = blk * 2048 + ch * 1024 + h2 * 512
                            nc.scalar.copy(virt[ct][:, c0:c0 + 512], mp[:])

                # ---- incremental GroupNorm stats (hide behind stage 1) --
                if do_stats and blk in (nblk // 2 - 1, nblk - 1):
                    h = 0 if blk == nblk // 2 - 1 else 1
                    for ct in range(2):
                        virt3v = virt[ct].rearrange(
                            "p (q b) -> p q b", b=B)
                        for b in range(B):
                            nc.vector.bn_stats(
                                out=stt2[ct][:, b, h, :],
                                in_=virt3v[:, h * 256:(h + 1) * 256, b])

        if dbg:
            for ct in range(2):
                nc.sync.dma_start(out=dbg_virt.ap()[ct],
                                  in_=virt[ct][:, 0:2048])

        # =================== STAGE 1.5: stats + collective ===============
        with ExitStack() as s15:
          if do_stats:
              st = s15.enter_context(tc.tile_pool(name="stsb", bufs=1))  # noqa
              stp = s15.enter_context(tc.tile_pool(name="stps", bufs=1,
                                                   space="PSUM"))
              packs = []
              for ct in range(2):
                  mv = st.tile([128, B, 2], F32, tag=f"mv{ct}", name=f"mv{ct}")
                  for b in range(B):
                      nc.vector.bn_aggr(out=mv[:, b, :],
                                        in_=stt2[ct][:, b, :, :])
                  pack = st.tile([128, 2 * B], F32, tag=f"pack{ct}")
                  # cols 0..63 = mean, 64..127 = E[x^2] = var + mean^2
                  nc.vector.tensor_copy(pack[:, 0:B], mv[:, :, 0])
                  nc.vector.tensor_tensor(out=pack[:, B:2 * B], in0=mv[:, :, 0],
                                          in1=mv[:, :, 0],
                                          op=mybir.AluOpType.mult)
                  nc.vector.tensor_tensor(out=pack[:, B:2 * B],
                                          in0=pack[:, B:2 * B], in1=mv[:, :, 1],
                                          op=mybir.AluOpType.add)
                  packs.append(pack)

              sum_ps = stp.tile([1, 2 * B], F32, tag="sump")
              for ct in range(2):
                  nc.tensor.matmul(sum_ps[:], ones1[:], packs[ct][:],
                                   start=(ct == 0), stop=(ct == 1))
              loc = st.tile([1, 2 * B], F32, tag="loc")
              nc.vector.tensor_copy(loc[:], sum_ps[:])

              cc_in = dram.tile([1, 2 * B], F32)
              cc_out = dram.tile([1, 2 * B], F32)
              nc.sync.dma_start(out=cc_in[:], in_=loc[:])
              if do_coll:
                  nc.gpsimd.collective_compute(
                      "AllReduce", mybir.AluOpType.add,
                      replica_groups=[list(range(NCORES))],
                      ins=[cc_in.opt()], outs=[cc_out.opt()])
                  bc_src = cc_out
              else:
                  bc_src = cc_in
              stat_bc = st.tile([128, 2 * B], F32, tag="statbc")
              nc.sync.dma_start(
                  out=stat_bc[:],
                  in_=bass.AP(tensor=bc_src.tensor, offset=bc_src.offset,
                              ap=[[0, 128], [1, 2 * B]]))

              # global mean/var/rstd  (sum over 2048 cells of (c, core))
              inv = 1.0 / (C * NCORES)
              mean_g = persist.tile([128, B], F32, tag="meang")
              nc.scalar.mul(mean_g[:], stat_bc[:, 0:B], inv)
              var_g = st.tile([128, B], F32, tag="varg")
              nc.scalar.mul(var_g[:], stat_bc[:, B:2 * B], inv)
              msq = st.tile([128, B], F32, tag="msq")
              nc.vector.tensor_tensor(out=msq[:], in0=mean_g[:], in1=mean_g[:],
                                      op=mybir.AluOpType.mult)
              nc.vector.tensor_tensor(out=var_g[:], in0=var_g[:], in1=msq[:],
                                      op=mybir.AluOpType.subtract)
              sd = st.tile([128, B], F32, tag="sd")
              eps_t = st.tile([128, 1], F32, tag="epst")
              nc.vector.memset(eps_t[:], EPS)
              nc.scalar.activation(sd[:], var_g[:], AF.Sqrt, bias=eps_t[:])
              rstd = st.tile([128, B], F32, tag="rstd")
              nc.vector.reciprocal(rstd[:], sd[:])

              # A[ct] = rstd * gamma_c ; Bb[ct] = beta_c - mean * A
              A_sb = [persist.tile([128, B], F32, tag=f"A{ct}", name=f"A{ct}") for ct in range(2)]
              B_sb = [persist.tile([128, B], F32, tag=f"Bb{ct}", name=f"Bb{ct}") for ct in range(2)]
              for ct in range(2):
                  nc.vector.tensor_scalar_mul(A_sb[ct][:], rstd[:], gam_sb[ct][:])
                  tmp = st.tile([128, B], F32, tag="tmpB")
                  nc.vector.tensor_tensor(out=tmp[:], in0=mean_g[:],
                                          in1=A_sb[ct][:],
                                          op=mybir.AluOpType.mult)
                  nc.scalar.activation(B_sb[ct][:], tmp[:], AF.Identity,
                                       bias=bet_sb[ct][:], scale=-1.0)

        # =================== STAGE 2 =====================================
        with ExitStack() as s2:
          if do_s2:
              p2 = s2.enter_context(tc.tile_pool(name="s2sb", bufs=3))
              pc = s2.enter_context(tc.tile_pool(name="pc", bufs=4, space="PSUM"))
              po = s2.enter_context(tc.tile_pool(name="po", bufs=4, space="PSUM"))

              virt3s = [virt[ct].rearrange("p (q b) -> p q b", b=B)
                        for ct in range(2)]
              for b in range(B):
                  rc = [p2.tile([128, LC], BF16, tag=f"rc{ct}", name=f"rc{ct}") for ct in range(2)]
                  for ct in range(2):
                      nc.scalar.activation(
                          rc[ct][:], virt3s[ct][:, :, b], AF.Relu,
                          bias=B_sb[ct][:, b:b + 1], scale=A_sb[ct][:, b:b + 1])
                  xf_t = [p2.tile([128, LC], BF16, tag=f"xt{ct}", name=f"xt{ct}") for ct in range(2)]
                  for ct in range(2):
                      nc.sync.dma_start(out=xf_t[ct][:], in_=xf.ap()[b, ct])
                  y = [p2.tile([128, LC], BF16, tag=f"y{ct}", name=f"y{ct}") for ct in range(2)]
                  for ot in range(2):
                      cp = pc.tile([128, LC], F32, tag="wcp")
                      for kt in range(2):
                          nc.tensor.matmul(cp[:],
                                           wc_sb[kt][:, ot * 128:(ot + 1) * 128],
                                           rc[kt][:],
                                           start=(kt == 0), stop=(kt == 1))
                      nc.vector.tensor_tensor(out=y[ot][:], in0=cp[:],
                                              in1=xf_t[ot][:],
                                              op=mybir.AluOpType.add)
                  for ot in range(2):
                      op_ = po.tile([128, LC], F32, tag="wop")
                      for kt in range(2):
                          nc.tensor.matmul(op_[:],
                                           wo_sb[kt][:, ot * 128:(ot + 1) * 128],
                                           y[kt][:],
                                           start=(kt == 0), stop=(kt == 1))
                      o_sb = p2.tile([128, LC], BF16, tag=f"o{ot}")
                      nc.scalar.copy(o_sb[:], op_[:])
                      nc.sync.dma_start(out=out.ap()[b, ot], in_=o_sb[:])

    nc.compile()
    return nc


def kernel(x, Wq, Wk, Wv, Wc, Wout, gamma, beta):
    x = np.asarray(x)
    Wq, Wk, Wv, Wc, Wout = map(np.asarray, (Wq, Wk, Wv, Wc, Wout))
    gamma, beta = np.asarray(gamma), np.asarray(beta)

    if "nc" not in _CACHE:
        _CACHE["nc"] = build()
    nc = _CACHE["nc"]

    # host weight prep
    M = (Wq.T @ Wk) / np.sqrt(np.float32(C))
    wg = np.ascontiguousarray(M.T.reshape(2, 128, C)).astype(BF16NP)
    wv = np.ascontiguousarray(Wv.T.reshape(2, 128, C)).astype(BF16NP)
    wc = np.ascontiguousarray(Wc.T.reshape(2, 128, C)).astype(BF16NP)
    wo = np.ascontiguousarray(Wout.T.reshape(2, 128, C)).astype(BF16NP)
    gb = np.stack([gamma.reshape(2, 128), beta.reshape(2, 128)]).astype(np.float32)
    gb = np.ascontiguousarray(gb)

    in_maps = []
    for m in range(NCORES):
        xs = np.ascontiguousarray(x[:, :, m * LC:(m + 1) * LC], dtype=np.float32)
        xb = np.ascontiguousarray(
            xs.transpose(1, 2, 0).reshape(2, 128, LC, B)).astype(BF16NP)
        xf = np.ascontiguousarray(xs.reshape(B, 2, 128, LC)).astype(BF16NP)
        in_maps.append(dict(xb=xb, xf=xf, wg=wg, wv=wv, wc=wc, wo=wo, gb=gb))

    import os
    trace = bool(int(os.environ.get("KERNEL_TRACE", "0")))
    res = run_bass_kernel_spmd(nc, in_maps, core_ids=list(range(NCORES)),
                               trace=trace)
    _CACHE["last_result"] = res
    shards = [res.results[m]["out"].astype(np.float32).reshape(B, C, LC)
              for m in range(NCORES)]
    return np.concatenate(shards, axis=2)



# revision 42
# speedup vs baseline: 1.3576x; 1.0620x over previous
"""Trainium2 Bass kernel for batch-attention block (B=64, C=256, L=4096).

Sequence-parallel over L across 8 cores (Lc=512 per core). Math:
  g = (WkT@Wq/sqrt(C))^T-conv of x ; attT[d,b,l] = sum_c g[c,d,l] x[c,b,l]
  e = exp(attT) (no max-subtract; values are O(+-8))
  s[b,l] = sum_d e[d,b,l] ; softmax normalization deferred: rs = 1/s folded
  into the PSUM evacuation of the mix matmul.
  vB[d,c,l] = (Wv x) computed directly in batch-major layout on PE.
  virt[c,b,l] = (sum_d vB[d,c,l] e[d,b,l]) * rs[b,l]   (kept in SBUF, bf16)
  rs is computed on a [128,16]-regrouped copy of s (128 DVE lanes for the
  8-cyc/elem divide), and applied via scalar PSUM->bf16 copy + DVE 2x mult.
  virt cols are (blk, ch, par, i, b); the host un-permutes output columns.
  GroupNorm raw sums per sample b via per-block ones-matmul c-folds of
  virt and virt^2 (accumulated in SBUF), AllReduce'd across cores, then
  rc = relu(virt * A_b + B_b), out = Wout-conv(x) + (Wout@Wc)-conv(rc)
  accumulated in one PSUM group (residual folded on host).
"""

import numpy as np
import ml_dtypes
from contextlib import ExitStack

from concourse import bass, bacc, tile, mybir
from concourse.bass_utils import run_bass_kernel_spmd

F32 = mybir.dt.float32
BF16 = mybir.dt.bfloat16
AF = mybir.ActivationFunctionType
BF16NP = ml_dtypes.bfloat16

NCORES = 8
B = 64
C = 256
L = 4096
LC = L // NCORES          # 512 positions per core
LB = 32                   # positions per block
NBLK = LC // LB           # 16
NPAIR = LB // 2           # 16 pairs per block
NGRP = 4                  # pairs per att-psum group
EPS = 1e-5

_CACHE = {}


# virt/out column order is (blk, ch, i, par, b) which equals natural
# l-order: l = ch*16 + i*2 + par for z = (ch, i, par).
PERM = np.arange(LC)


def build(nblk=NBLK, do_stats=True, do_coll=True, do_s2=True, dbg=False):
    nc = bacc.Bacc("TRN2", target_bir_lowering=False, debug=False,
                   num_devices=NCORES)
    if dbg:
        dbg_s = nc.dram_tensor("dbg_s", [2, 2048], BF16, kind="ExternalOutput")
        dbg_s128 = nc.dram_tensor("dbg_s128", [128, 16], BF16,
                                  kind="ExternalOutput")
        dbg_rs128 = nc.dram_tensor("dbg_rs128", [128, 16], BF16,
                                   kind="ExternalOutput")
        dbg_rsbc = nc.dram_tensor("dbg_rsbc", [128, 2048], BF16,
                                  kind="ExternalOutput")
        dbg_e = nc.dram_tensor("dbg_e", [128, 2048], BF16,
                               kind="ExternalOutput")
        dbg_virt = nc.dram_tensor("dbg_virt", [2, 128, 2048], BF16,
                                  kind="ExternalOutput")

    xb = nc.dram_tensor("xb", [2, 128, LC, B], BF16, kind="ExternalInput")
    xf = nc.dram_tensor("xf", [B, 2, 128, LC], BF16, kind="ExternalInput")
    wg = nc.dram_tensor("wg", [2, 128, C], BF16, kind="ExternalInput")
    wv = nc.dram_tensor("wv", [2, 128, C], BF16, kind="ExternalInput")
    wc = nc.dram_tensor("wc", [2, 128, C], BF16, kind="ExternalInput")  # Wout@Wc fused
    wo = nc.dram_tensor("wo", [2, 128, C], BF16, kind="ExternalInput")
    gb = nc.dram_tensor("gb", [2, 2, 128], F32, kind="ExternalInput")
    out = nc.dram_tensor("out", [B, 2, 128, LC], BF16, kind="ExternalOutput")

    with tile.TileContext(nc) as tc, ExitStack() as top:
        persist = top.enter_context(tc.tile_pool(name="persist", bufs=1))
        dram = top.enter_context(tc.tile_pool(name="dram", bufs=1, space="DRAM"))

        # ---- persistent SBUF: weights, virt, constants -------------------
        wg_sb, wv_sb, wc_sb, wo_sb = [], [], [], []
        for nm, dr, lst in (("wg", wg, wg_sb), ("wv", wv, wv_sb),
                            ("wc", wc, wc_sb), ("wo", wo, wo_sb)):
            for ct in range(2):
                t = persist.tile([128, C], BF16, tag=f"{nm}{ct}")
                nc.sync.dma_start(out=t[:], in_=dr.ap()[ct])
                lst.append(t)

        gam_sb, bet_sb = [], []
        for ct in range(2):
            t = persist.tile([128, 1], F32, tag=f"gam{ct}")
            nc.sync.dma_start(
                out=t[:], in_=gb.ap()[0, ct].rearrange("(p one) -> p one", one=1))
            gam_sb.append(t)
            t = persist.tile([128, 1], F32, tag=f"bet{ct}")
            nc.sync.dma_start(
                out=t[:], in_=gb.ap()[1, ct].rearrange("(p one) -> p one", one=1))
            bet_sb.append(t)

        ones2 = persist.tile([128, 2], BF16, tag="ones2")
        nc.vector.memset(ones2[:], 0.0)
        nc.vector.memset(ones2[0:64, 0:1], 1.0)
        nc.vector.memset(ones2[64:128, 1:2], 1.0)
        ones1 = persist.tile([128, 1], F32, tag="ones1")
        nc.vector.memset(ones1[:], 1.0)

        # rs broadcast tiles with zeroed junk quadrants (rows par != col
        # par); zeros persist because the DMAs only write matched slots.
        rsJ = [persist.tile([128, NPAIR * 128], BF16, tag=f"rsJ{k}",
                            name=f"rsJ{k}")
               for k in range(2)]
        for k in range(2):
            nc.vector.memset(rsJ[k][:], 0.0)

        virt = [persist.tile([128, B * LC], BF16, tag=f"virt{ct}", name=f"virt{ct}")
                for ct in range(2)]

        # GroupNorm raw-sum accumulators, cols (z=32, b=64) summed over
        # blocks; single-partition tiles (DVE cost is FD-bound anyway).
        ones1b = persist.tile([128, 1], BF16, tag="ones1b")
        nc.vector.memset(ones1b[:], 1.0)
        acc1 = persist.tile([1, 2048], F32, tag="acc1")
        acc2 = persist.tile([1, 2048], F32, tag="acc2")
        nc.vector.memset(acc1[:], 0.0)
        nc.vector.memset(acc2[:], 0.0)

        # softmax-denominator plumbing: s [2, 2048] per block -> regrouped
        # [128, 16] so the 8-cyc/elem divide uses all 128 DVE lanes.
        s_dr = dram.tile([NBLK, 2, NPAIR * 128], BF16)
        rs2_dr = dram.tile([NBLK, 128, NPAIR], BF16)

        # =================== STAGE 1 =====================================
        with ExitStack() as s1:
            sp = s1.enter_context(tc.tile_pool(name="s1sb", bufs=2))
            sp1 = s1.enter_context(tc.tile_pool(name="s1sb1", bufs=1))
            pw = s1.enter_context(tc.tile_pool(name="pw", bufs=3, space="PSUM"))
            pm = s1.enter_context(tc.tile_pool(name="pm", bufs=2, space="PSUM"))

            for blk in range(nblk):
                # ---- load x block (bf16, c-major, cols = (l, b)) --------
                xbf = [sp.tile([128, LB * B], BF16, tag=f"xbf{ct}", name=f"xbf{ct}")
                       for ct in range(2)]
                for ct in range(2):
                    nc.sync.dma_start(
                        out=xbf[ct][:],
                        in_=xb.ap()[ct, :, blk * LB:(blk + 1) * LB, :])

                # ---- g conv: g = lhsT_g.T @ x ---------------------------
                g_sb = [sp1.tile([128, LB * B], BF16, tag=f"g{ct}", name=f"g{ct}")
                        for ct in range(2)]
                for ct1 in range(2):
                    for ch in range(2):
                        gp = pw.tile([128, 1024], F32, tag="w", name="gp")
                        for h in range(2):
                            for ct2 in range(2):
                                nc.tensor.matmul(
                                    gp[:, h * 512:(h + 1) * 512],
                                    wg_sb[ct2][:, ct1 * 128:(ct1 + 1) * 128],
                                    xbf[ct2][:, ch * 1024 + h * 512:
                                              ch * 1024 + (h + 1) * 512],
                                    start=(ct2 == 0), stop=(ct2 == 1))
                        nc.scalar.copy(g_sb[ct1][:, ch * 1024:(ch + 1) * 1024],
                                       gp[:])

                # ---- att (paired, junk halves) + exp + sums -------------
                e_sb = sp.tile([128, NPAIR * 128], BF16, tag="e")
                for grp in range(2):
                    ap_ = pw.tile([128, 1024], F32, tag="w", name="attp")
                    for pi in range(8):
                        p = grp * 8 + pi
                        for kt in range(2):
                            nc.tensor.matmul(
                                ap_[:, pi * 128:(pi + 1) * 128],
                                g_sb[kt][:, p * 128:(p + 1) * 128],
                                xbf[kt][:, p * 128:(p + 1) * 128],
                                start=(kt == 0), stop=(kt == 1))
                    nc.scalar.activation(
                        e_sb[:, grp * 1024:(grp + 1) * 1024], ap_[:], AF.Exp)
                    s_ps = pw.tile([2, 1024], F32, tag="w", name="sp")
                    for h in range(2):
                        nc.tensor.matmul(
                            s_ps[:, h * 512:(h + 1) * 512], ones2[:],
                            e_sb[:, grp * 1024 + h * 512:
                                 grp * 1024 + (h + 1) * 512],
                            start=True, stop=True)
                    # s -> SBUF bf16 -> DRAM; divide happens after the
                    # [128, 16] regroup so it uses all DVE lanes.
                    s_sb = sp.tile([2, 1024], BF16, tag="ssb")
                    nc.scalar.copy(s_sb[:], s_ps[:])
                    nc.sync.dma_start(
                        out=s_dr[blk][:, grp * 1024:(grp + 1) * 1024],
                        in_=s_sb[:])

                # s -> [128, 16] (part = par*64 + pair*4 + b//16, col =
                # b%16), 1/s, -> DRAM laid out [par, pair, b] so the
                # broadcast reads are inner-contiguous.
                s128 = sp.tile([128, NPAIR], BF16, tag="s128")
                for par in range(2):
                    nc.sync.dma_start(
                        out=s128[par * B:(par + 1) * B, :],
                        in_=bass.AP(tensor=s_dr.tensor,
                                    offset=s_dr.offset + blk * 2 * 2048
                                    + par * 2112,
                                    ap=[[128, NPAIR], [16, 4], [1, 16]]))
                rs128 = sp.tile([128, NPAIR], BF16, tag="rs128")
                with nc.allow_low_precision(reason="softmax rs in bf16"):
                    nc.vector.reciprocal(rs128[:], s128[:])
                nc.sync.dma_start(out=rs2_dr[blk], in_=rs128[:])
                rs_bc = rsJ[blk % 2]
                rsv = rs_bc.rearrange("p (pr two b) -> p pr two b",
                                      two=2, b=B)
                for rp in range(2):
                    src = bass.AP(
                        tensor=rs2_dr.tensor,
                        offset=rs2_dr.offset + blk * 128 * NPAIR
                        + rp * NPAIR * B,
                        ap=[[0, 64], [B, NPAIR], [1, B]])
                    nc.sync.dma_start(
                        out=rsv[rp * 64:(rp + 1) * 64, :, rp, :], in_=src)
                # e_n = e * rs in place (junk quadrants multiplied by 0);
                # the sums matmuls read e_sb before this, so Tile orders it.
                e_n = e_sb
                nc.vector.tensor_tensor(out=e_n[:], in0=e_sb[:],
                                        in1=rs_bc[:],
                                        op=mybir.AluOpType.mult)
                if dbg and blk == 0:
                    nc.sync.dma_start(out=dbg_s.ap(), in_=s_dr[0])
                    nc.sync.dma_start(out=dbg_s128.ap(), in_=s128[:])
                    nc.sync.dma_start(out=dbg_rs128.ap(), in_=rs128[:])
                    nc.sync.dma_start(out=dbg_rsbc.ap(), in_=e_n[:])
                    nc.sync.dma_start(out=dbg_e.ap(), in_=e_sb[:])

                # ---- vB: per pair one (128,128,256) matmul --------------
                vb_sb = sp1.tile([128, NPAIR * C], BF16, tag="vb")
                for ph in range(NPAIR // 4):
                    vp = pw.tile([128, 1024], F32, tag="w", name="vbp")
                    for pi in range(4):
                        p = ph * 4 + pi
                        for kt in range(2):
                            nc.tensor.matmul(
                                vp[:, pi * 256:(pi + 1) * 256],
                                xbf[kt][:, p * 128:(p + 1) * 128],
                                wv_sb[kt][:],
                                start=(kt == 0), stop=(kt == 1))
                    if ph < 2:
                        nc.scalar.copy(vb_sb[:, ph * 1024:(ph + 1) * 1024],
                                       vp[:])
                    else:
                        nc.vector.tensor_copy(
                            vb_sb[:, ph * 1024:(ph + 1) * 1024], vp[:])

                # ---- mix: one [128,128]x[128,128] MM per (pair, ct) -----
                # e_n junk quadrants are zero, so a full-width MM gives
                # both parities; virt cols = (blk, ch, i, par, b) =
                # natural l-order (host permutation is identity).
                for ct in range(2):
                    for ch in range(2):
                        for h2 in range(2):
                            mp = pm.tile([128, 512], F32, tag="mixp")
                            for i4 in range(4):
                                i = h2 * 4 + i4
                                p = ch * 8 + i
                                nc.tensor.matmul(
                                    mp[:, i4 * 128:(i4 + 1) * 128],
                                    vb_sb[:, p * 256 + ct * 128:
                                          p * 256 + (ct + 1) * 128],
                                    e_n[:, p * 128:(p + 1) * 128],
                                    start=True, stop=True)
                            c0 = blk * 2048 + ch * 1024 + h2 * 512
                            nc.scalar.copy(virt[ct][:, c0:c0 + 512], mp[:])

                # ---- GroupNorm raw sums, spread per block ----------------
                # S1: ones^T @ virt chunk -> [1, 512] c-fold, added to acc1.
                # S2: same on sq = virt^2 (DVE 2x mult), added to acc2.
                if do_stats:
                    for chunk in range(4):
                        cc0 = blk * 2048 + chunk * 512
                        f1 = pm.tile([1, 512], F32, tag="mixp", name="f1")
                        for ct in range(2):
                            nc.tensor.matmul(
                                f1[:], ones1b[:], virt[ct][:, cc0:cc0 + 512],
                                start=(ct == 0), stop=(ct == 1))
                        nc.vector.tensor_tensor(
                            out=acc1[:, chunk * 512:(chunk + 1) * 512],
                            in0=f1[:],
                            in1=acc1[:, chunk * 512:(chunk + 1) * 512],
                            op=mybir.AluOpType.add)
                        f2 = pm.tile([1, 512], F32, tag="mixp", name="f2")
                        for ct in range(2):
                            sq = sp.tile([128, 512], BF16, tag="sq")
                            nc.vector.tensor_tensor(
                                out=sq[:], in0=virt[ct][:, cc0:cc0 + 512],
                                in1=virt[ct][:, cc0:cc0 + 512],
                                op=mybir.AluOpType.mult)
                            nc.tensor.matmul(
                                f2[:], ones1b[:], sq[:],
                                start=(ct == 0), stop=(ct == 1))
                        nc.vector.tensor_tensor(
                            out=acc2[:, chunk * 512:(chunk + 1) * 512],
                            in0=f2[:],
                            in1=acc2[:, chunk * 512:(chunk + 1) * 512],
                            op=mybir.AluOpType.add)

        if dbg:
            for ct in range(2):
                nc.sync.dma_start(out=dbg_virt.ap()[ct],
                                  in_=virt[ct][:, 0:2048])

        # =================== STAGE 1.5: stats + collective ===============
        with ExitStack() as s15:
          if do_stats:
              st = s15.enter_context(tc.tile_pool(name="stsb", bufs=1))  # noqa
              # fold acc (z, b) -> per-b raw sums: [1, 2B] = (S1 | S2)
              loc = st.tile([1, 2 * B], F32, tag="loc")
              nc.vector.tensor_reduce(
                  out=loc[:, 0:B],
                  in_=acc1.rearrange("p (z b) -> p b z", b=B),
                  axis=mybir.AxisListType.X, op=mybir.AluOpType.add)
              nc.vector.tensor_reduce(
                  out=loc[:, B:2 * B],
                  in_=acc2.rearrange("p (z b) -> p b z", b=B),
                  axis=mybir.AxisListType.X, op=mybir.AluOpType.add)

              cc_in = dram.tile([1, 2 * B], F32)
              cc_out = dram.tile([1, 2 * B], F32)
              nc.sync.dma_start(out=cc_in[:], in_=loc[:])
              if do_coll:
                  nc.gpsimd.collective_compute(
                      "AllReduce", mybir.AluOpType.add,
                      replica_groups=[list(range(NCORES))],
                      ins=[cc_in.opt()], outs=[cc_out.opt()])
                  bc_src = cc_out
              else:
                  bc_src = cc_in
              stat_bc = st.tile([128, 2 * B], F32, tag="statbc")
              nc.sync.dma_start(
                  out=stat_bc[:],
                  in_=bass.AP(tensor=bc_src.tensor, offset=bc_src.offset,
                              ap=[[0, 128], [1, 2 * B]]))

              # global mean/var/rstd  (raw sums over (C, L) across cores)
              inv = 1.0 / (C * L)
              mean_g = persist.tile([128, B], F32, tag="meang")
              nc.scalar.mul(mean_g[:], stat_bc[:, 0:B], inv)
              var_g = st.tile([128, B], F32, tag="varg")
              nc.scalar.mul(var_g[:], stat_bc[:, B:2 * B], inv)
              msq = st.tile([128, B], F32, tag="msq")
              nc.vector.tensor_tensor(out=msq[:], in0=mean_g[:], in1=mean_g[:],
                                      op=mybir.AluOpType.mult)
              nc.vector.tensor_tensor(out=var_g[:], in0=var_g[:], in1=msq[:],
                                      op=mybir.AluOpType.subtract)
              sd = st.tile([128, B], F32, tag="sd")
              eps_t = st.tile([128, 1], F32, tag="epst")
              nc.vector.memset(eps_t[:], EPS)
              nc.scalar.activation(sd[:], var_g[:], AF.Sqrt, bias=eps_t[:])
              rstd = st.tile([128, B], F32, tag="rstd")
              nc.vector.reciprocal(rstd[:], sd[:])

              # A[ct] = rstd * gamma_c ; Bb[ct] = beta_c - mean * A
              A_sb = [persist.tile([128, B], F32, tag=f"A{ct}", name=f"A{ct}") for ct in range(2)]
              B_sb = [persist.tile([128, B], F32, tag=f"Bb{ct}", name=f"Bb{ct}") for ct in range(2)]
              for ct in range(2):
                  nc.vector.tensor_scalar_mul(A_sb[ct][:], rstd[:], gam_sb[ct][:])
                  tmp = st.tile([128, B], F32, tag="tmpB")
                  nc.vector.tensor_tensor(out=tmp[:], in0=mean_g[:],
                                          in1=A_sb[ct][:],
                                          op=mybir.AluOpType.mult)
                  nc.scalar.activation(B_sb[ct][:], tmp[:], AF.Identity,
                                       bias=bet_sb[ct][:], scale=-1.0)

        # =================== STAGE 2 =====================================
        # out = Wout x + (Wout Wc) relu(A virt + B): the residual add is
        # folded into the PSUM accumulation (wc_sb holds host-fused Wout@Wc).
        with ExitStack() as s2:
          if do_s2:
              p2 = s2.enter_context(tc.tile_pool(name="s2sb", bufs=3))
              po = s2.enter_context(tc.tile_pool(name="po", bufs=6, space="PSUM"))

              virt3s = [virt[ct].rearrange("p (q b) -> p q b", b=B)
                        for ct in range(2)]
              for b in range(B):
                  rc = [p2.tile([128, LC], BF16, tag=f"rc{ct}", name=f"rc{ct}") for ct in range(2)]
                  for ct in range(2):
                      nc.scalar.activation(
                          rc[ct][:], virt3s[ct][:, :, b], AF.Relu,
                          bias=B_sb[ct][:, b:b + 1], scale=A_sb[ct][:, b:b + 1])
                  xf_t = [p2.tile([128, LC], BF16, tag=f"xt{ct}", name=f"xt{ct}") for ct in range(2)]
                  for ct in range(2):
                      nc.sync.dma_start(out=xf_t[ct][:], in_=xf.ap()[b, ct])
                  for ot in range(2):
                      op_ = po.tile([128, LC], F32, tag="wop")
                      nc.tensor.matmul(op_[:], wo_sb[0][:, ot * 128:(ot + 1) * 128],
                                       xf_t[0][:], start=True, stop=False)
                      nc.tensor.matmul(op_[:], wo_sb[1][:, ot * 128:(ot + 1) * 128],
                                       xf_t[1][:], start=False, stop=False)
                      nc.tensor.matmul(op_[:], wc_sb[0][:, ot * 128:(ot + 1) * 128],
                                       rc[0][:], start=False, stop=False)
                      nc.tensor.matmul(op_[:], wc_sb[1][:, ot * 128:(ot + 1) * 128],
                                       rc[1][:], start=False, stop=True)
                      o_sb = p2.tile([128, LC], BF16, tag=f"o{ot}")
                      if ot == 0:
                          nc.scalar.copy(o_sb[:], op_[:])
                      else:
                          nc.vector.tensor_copy(o_sb[:], op_[:])
                      nc.sync.dma_start(out=out.ap()[b, ot], in_=o_sb[:])

    nc.compile()
    return nc


def kernel(x, Wq, Wk, Wv, Wc, Wout, gamma, beta):
    x = np.asarray(x)
    Wq, Wk, Wv, Wc, Wout = map(np.asarray, (Wq, Wk, Wv, Wc, Wout))
    gamma, beta = np.asarray(gamma), np.asarray(beta)

    if "nc" not in _CACHE:
        _CACHE["nc"] = build()
    nc = _CACHE["nc"]

    # host weight prep
    M = (Wq.T @ Wk) / np.sqrt(np.float32(C))
    Wf = (Wout @ Wc).astype(np.float32)   # fused: out = Wout x + Wf relu(..)
    wg = np.ascontiguousarray(M.T.reshape(2, 128, C)).astype(BF16NP)
    wv = np.ascontiguousarray(Wv.T.reshape(2, 128, C)).astype(BF16NP)
    wc = np.ascontiguousarray(Wf.T.reshape(2, 128, C)).astype(BF16NP)
    wo = np.ascontiguousarray(Wout.T.reshape(2, 128, C)).astype(BF16NP)
    gb = np.stack([gamma.reshape(2, 128), beta.reshape(2, 128)]).astype(np.float32)
    gb = np.ascontiguousarray(gb)

    in_maps = []
    for m in range(NCORES):
        xs = np.ascontiguousarray(x[:, :, m * LC:(m + 1) * LC], dtype=np.float32)
        xb = np.ascontiguousarray(
            xs.transpose(1, 2, 0).reshape(2, 128, LC, B)).astype(BF16NP)
        xf = np.ascontiguousarray(xs.reshape(B, 2, 128, LC)).astype(BF16NP)
        in_maps.append(dict(xb=xb, xf=xf, wg=wg, wv=wv, wc=wc, wo=wo, gb=gb))

    import os
    trace = bool(int(os.environ.get("KERNEL_TRACE", "0")))
    res = run_bass_kernel_spmd(nc, in_maps, core_ids=list(range(NCORES)),
                               trace=trace)
    _CACHE["last_result"] = res
    shards = [res.results[m]["out"].astype(np.float32).reshape(B, C, LC)
              for m in range(NCORES)]
    return np.concatenate(shards, axis=2)



# revision 47
# speedup vs baseline: 1.4168x; 1.0436x over previous
"""Trainium2 Bass kernel for batch-attention block (B=64, C=256, L=4096).

Sequence-parallel over L across 8 cores (Lc=512 per core). Math:
  g = (WkT@Wq/sqrt(C))^T-conv of x ; attT[d,b,l] = sum_c g[c,d,l] x[c,b,l]
  e = exp(attT) (no max-subtract; values are O(+-8))
  s[b,l] = sum_d e[d,b,l] ; softmax normalization deferred: rs = 1/s folded
  into the PSUM evacuation of the mix matmul.
  vB[d,c,l] = (Wv x) computed directly in batch-major layout on PE.
  virt[c,b,l] = (sum_d vB[d,c,l] e[d,b,l]) * rs[b,l]   (kept in SBUF, bf16)
  rs is computed on a [128,16]-regrouped copy of s (128 DVE lanes for the
  8-cyc/elem divide), and applied via scalar PSUM->bf16 copy + DVE 2x mult.
  virt cols are (blk, ch, par, i, b); the host un-permutes output columns.
  GroupNorm raw sums per sample b via per-block ones-matmul c-folds of
  virt and virt^2 (accumulated in SBUF), AllReduce'd across cores, then
  rc = relu(virt * A_b + B_b), out = Wout-conv(x) + (Wout@Wc)-conv(rc)
  accumulated in one PSUM group (residual folded on host).
"""

import numpy as np
import ml_dtypes
from contextlib import ExitStack

from concourse import bass, bacc, tile, mybir
from concourse.bass_utils import run_bass_kernel_spmd

F32 = mybir.dt.float32
BF16 = mybir.dt.bfloat16
AF = mybir.ActivationFunctionType
BF16NP = ml_dtypes.bfloat16

NCORES = 8
B = 64
C = 256
L = 4096
LC = L // NCORES          # 512 positions per core
LB = 32                   # positions per block
NBLK = LC // LB           # 16
NPAIR = LB // 2           # 16 pairs per block
NGRP = 4                  # pairs per att-psum group
EPS = 1e-5

_CACHE = {}


# virt/out column order is (blk, ch, i, par, b) which equals natural
# l-order: l = ch*16 + i*2 + par for z = (ch, i, par).
PERM = np.arange(LC)


def build(nblk=NBLK, do_stats=True, do_coll=True, do_s2=True, dbg=False):
    nc = bacc.Bacc("TRN2", target_bir_lowering=False, debug=False,
                   num_devices=NCORES)
    if dbg:
        dbg_s = nc.dram_tensor("dbg_s", [2, 2048], BF16, kind="ExternalOutput")
        dbg_s128 = nc.dram_tensor("dbg_s128", [128, 16], BF16,
                                  kind="ExternalOutput")
        dbg_rs128 = nc.dram_tensor("dbg_rs128", [128, 16], BF16,
                                   kind="ExternalOutput")
        dbg_rsbc = nc.dram_tensor("dbg_rsbc", [128, 2048], BF16,
                                  kind="ExternalOutput")
        dbg_e = nc.dram_tensor("dbg_e", [128, 2048], BF16,
                               kind="ExternalOutput")
        dbg_virt = nc.dram_tensor("dbg_virt", [2, 128, 2048], BF16,
                                  kind="ExternalOutput")

    xb = nc.dram_tensor("xb", [2, 128, LC, B], BF16, kind="ExternalInput")
    xf = nc.dram_tensor("xf", [B, 2, 128, LC], BF16, kind="ExternalInput")
    wg = nc.dram_tensor("wg", [2, 128, C], BF16, kind="ExternalInput")
    wv = nc.dram_tensor("wv", [2, 128, C], BF16, kind="ExternalInput")
    wc = nc.dram_tensor("wc", [2, 128, C], BF16, kind="ExternalInput")  # Wout@Wc fused
    wo = nc.dram_tensor("wo", [2, 128, C], BF16, kind="ExternalInput")
    gb = nc.dram_tensor("gb", [2, 2, 128], F32, kind="ExternalInput")
    out = nc.dram_tensor("out", [B, 2, 128, LC], BF16, kind="ExternalOutput")

    with tile.TileContext(nc) as tc, ExitStack() as top:
        persist = top.enter_context(tc.tile_pool(name="persist", bufs=1))
        dram = top.enter_context(tc.tile_pool(name="dram", bufs=1, space="DRAM"))

        # ---- persistent SBUF: weights, virt, constants -------------------
        wg_sb, wv_sb, wc_sb, wo_sb = [], [], [], []
        for nm, dr, lst in (("wg", wg, wg_sb), ("wv", wv, wv_sb),
                            ("wc", wc, wc_sb), ("wo", wo, wo_sb)):
            for ct in range(2):
                t = persist.tile([128, C], BF16, tag=f"{nm}{ct}")
                nc.sync.dma_start(out=t[:], in_=dr.ap()[ct])
                lst.append(t)

        gam_sb, bet_sb = [], []
        for ct in range(2):
            t = persist.tile([128, 1], F32, tag=f"gam{ct}")
            nc.sync.dma_start(
                out=t[:], in_=gb.ap()[0, ct].rearrange("(p one) -> p one", one=1))
            gam_sb.append(t)
            t = persist.tile([128, 1], F32, tag=f"bet{ct}")
            nc.sync.dma_start(
                out=t[:], in_=gb.ap()[1, ct].rearrange("(p one) -> p one", one=1))
            bet_sb.append(t)

        ones2 = persist.tile([128, 2], BF16, tag="ones2")
        nc.vector.memset(ones2[:], 0.0)
        nc.vector.memset(ones2[0:64, 0:1], 1.0)
        nc.vector.memset(ones2[64:128, 1:2], 1.0)
        ones1 = persist.tile([128, 1], F32, tag="ones1")
        nc.vector.memset(ones1[:], 1.0)

        # rs broadcast tiles with zeroed junk quadrants (rows par != col
        # par); zeros persist because the DMAs only write matched slots.
        rsJ = [persist.tile([128, NPAIR * 128], BF16, tag=f"rsJ{k}",
                            name=f"rsJ{k}")
               for k in range(2)]
        for k in range(2):
            nc.vector.memset(rsJ[k][:], 0.0)

        virt = [persist.tile([128, B * LC], BF16, tag=f"virt{ct}", name=f"virt{ct}")
                for ct in range(2)]

        # GroupNorm raw-sum accumulators, cols (z=32, b=64) summed over
        # blocks; single-partition tiles (DVE cost is FD-bound anyway).
        ones1b = persist.tile([128, 1], BF16, tag="ones1b")
        nc.vector.memset(ones1b[:], 1.0)
        acc1 = persist.tile([1, 2048], BF16, tag="acc1")
        acc2 = persist.tile([1, 2048], BF16, tag="acc2")
        nc.vector.memset(acc1[:], 0.0)
        nc.vector.memset(acc2[:], 0.0)

        # softmax-denominator plumbing: s [2, 2048] per block -> regrouped
        # [128, 16] so the 8-cyc/elem divide uses all 128 DVE lanes.
        s_dr = dram.tile([NBLK, 2, NPAIR * 128], BF16)
        rs2_dr = dram.tile([NBLK, 128, NPAIR], BF16)

        # =================== STAGE 1 =====================================
        with ExitStack() as s1:
            sp = s1.enter_context(tc.tile_pool(name="s1sb", bufs=2))
            sp1 = s1.enter_context(tc.tile_pool(name="s1sb1", bufs=1))
            pw = s1.enter_context(tc.tile_pool(name="pw", bufs=3, space="PSUM"))
            pm = s1.enter_context(tc.tile_pool(name="pm", bufs=2, space="PSUM"))

            def mix_and_stats(blk, vb_sb, e_n):
                # ---- mix: one [128,128]x[128,128] MM per (pair, ct) -----
                # e_n junk quadrants are zero, so a full-width MM gives
                # both parities; virt cols = (blk, ch, i, par, b) =
                # natural l-order (host permutation is identity).
                for ct in range(2):
                    for ch in range(2):
                        for h2 in range(2):
                            mp = pm.tile([128, 512], F32, tag="mixp",
                                         name="mp")
                            for i4 in range(4):
                                i = h2 * 4 + i4
                                p = ch * 8 + i
                                nc.tensor.matmul(
                                    mp[:, i4 * 128:(i4 + 1) * 128],
                                    vb_sb[:, p * 256 + ct * 128:
                                          p * 256 + (ct + 1) * 128],
                                    e_n[:, p * 128:(p + 1) * 128],
                                    start=True, stop=True)
                            c0 = blk * 2048 + ch * 1024 + h2 * 512
                            nc.scalar.copy(virt[ct][:, c0:c0 + 512], mp[:])

                # ---- GroupNorm raw sums: c-folds of virt and virt^2 -----
                if do_stats:
                    for chunk in range(4):
                        cc0 = blk * 2048 + chunk * 512
                        f1 = pm.tile([1, 512], F32, tag="mixp", name="f1")
                        for ct in range(2):
                            nc.tensor.matmul(
                                f1[:], ones1b[:], virt[ct][:, cc0:cc0 + 512],
                                start=(ct == 0), stop=(ct == 1))
                        with nc.allow_low_precision(reason="stat acc bf16"):
                            nc.vector.tensor_tensor(
                                out=acc1[:, chunk * 512:(chunk + 1) * 512],
                                in0=f1[:],
                                in1=acc1[:, chunk * 512:(chunk + 1) * 512],
                                op=mybir.AluOpType.add)
                        f2 = pm.tile([1, 512], F32, tag="mixp", name="f2")
                        for ct in range(2):
                            sq = sp.tile([128, 512], BF16, tag="sq")
                            nc.vector.tensor_tensor(
                                out=sq[:], in0=virt[ct][:, cc0:cc0 + 512],
                                in1=virt[ct][:, cc0:cc0 + 512],
                                op=mybir.AluOpType.mult)
                            nc.tensor.matmul(
                                f2[:], ones1b[:], sq[:],
                                start=(ct == 0), stop=(ct == 1))
                        with nc.allow_low_precision(reason="stat acc bf16"):
                            nc.vector.tensor_tensor(
                                out=acc2[:, chunk * 512:(chunk + 1) * 512],
                                in0=f2[:],
                                in1=acc2[:, chunk * 512:(chunk + 1) * 512],
                                op=mybir.AluOpType.add)

            prev = None  # (blk, vb_sb, e_n) one block behind
            for blk in range(nblk):
                # ---- load x block (bf16, c-major, cols = (l, b)) --------
                xbf = [sp.tile([128, LB * B], BF16, tag=f"xbf{ct}", name=f"xbf{ct}")
                       for ct in range(2)]
                for ct in range(2):
                    nc.sync.dma_start(
                        out=xbf[ct][:],
                        in_=xb.ap()[ct, :, blk * LB:(blk + 1) * LB, :])

                # ---- g conv: g = lhsT_g.T @ x ---------------------------
                g_sb = [sp1.tile([128, LB * B], BF16, tag=f"g{ct}", name=f"g{ct}")
                        for ct in range(2)]
                for ct1 in range(2):
                    for ch in range(2):
                        gp = pw.tile([128, 1024], F32, tag="w", name="gp")
                        for h in range(2):
                            for ct2 in range(2):
                                nc.tensor.matmul(
                                    gp[:, h * 512:(h + 1) * 512],
                                    wg_sb[ct2][:, ct1 * 128:(ct1 + 1) * 128],
                                    xbf[ct2][:, ch * 1024 + h * 512:
                                              ch * 1024 + (h + 1) * 512],
                                    start=(ct2 == 0), stop=(ct2 == 1))
                        nc.scalar.copy(g_sb[ct1][:, ch * 1024:(ch + 1) * 1024],
                                       gp[:])

                # ---- att (paired, junk halves) + exp + sums -------------
                e_sb = sp.tile([128, NPAIR * 128], BF16, tag="e")
                for grp in range(2):
                    ap_ = pw.tile([128, 1024], F32, tag="w", name="attp")
                    for pi in range(8):
                        p = grp * 8 + pi
                        for kt in range(2):
                            nc.tensor.matmul(
                                ap_[:, pi * 128:(pi + 1) * 128],
                                g_sb[kt][:, p * 128:(p + 1) * 128],
                                xbf[kt][:, p * 128:(p + 1) * 128],
                                start=(kt == 0), stop=(kt == 1))
                    nc.scalar.activation(
                        e_sb[:, grp * 1024:(grp + 1) * 1024], ap_[:], AF.Exp)
                    s_ps = pw.tile([2, 1024], F32, tag="w", name="sp")
                    for h in range(2):
                        nc.tensor.matmul(
                            s_ps[:, h * 512:(h + 1) * 512], ones2[:],
                            e_sb[:, grp * 1024 + h * 512:
                                 grp * 1024 + (h + 1) * 512],
                            start=True, stop=True)
                    # s -> SBUF bf16 -> DRAM; divide happens after the
                    # [128, 16] regroup so it uses all DVE lanes.
                    s_sb = sp.tile([2, 1024], BF16, tag="ssb")
                    nc.scalar.copy(s_sb[:], s_ps[:])
                    nc.sync.dma_start(
                        out=s_dr[blk][:, grp * 1024:(grp + 1) * 1024],
                        in_=s_sb[:])

                # ---- vB: per pair one (128,128,256) matmul --------------
                vb_sb = sp.tile([128, NPAIR * C], BF16, tag="vb")
                for ph in range(NPAIR // 4):
                    vp = pw.tile([128, 1024], F32, tag="w", name="vbp")
                    for pi in range(4):
                        p = ph * 4 + pi
                        for kt in range(2):
                            nc.tensor.matmul(
                                vp[:, pi * 256:(pi + 1) * 256],
                                xbf[kt][:, p * 128:(p + 1) * 128],
                                wv_sb[kt][:],
                                start=(kt == 0), stop=(kt == 1))
                    if ph < 2:
                        nc.scalar.copy(vb_sb[:, ph * 1024:(ph + 1) * 1024],
                                       vp[:])
                    else:
                        nc.vector.tensor_copy(
                            vb_sb[:, ph * 1024:(ph + 1) * 1024], vp[:])

                # ---- previous block's mix + stats (pipelined so the PE
                # never waits on this block's rs DMA chain) --------------
                if prev is not None:
                    mix_and_stats(*prev)

                # s -> [128, 16] (part = par*64 + pair*4 + b//16, col =
                # b%16), 1/s, -> DRAM laid out [par, pair, b] so the
                # broadcast reads are inner-contiguous.
                s128 = sp.tile([128, NPAIR], BF16, tag="s128")
                for par in range(2):
                    nc.sync.dma_start(
                        out=s128[par * B:(par + 1) * B, :],
                        in_=bass.AP(tensor=s_dr.tensor,
                                    offset=s_dr.offset + blk * 2 * 2048
                                    + par * 2112,
                                    ap=[[128, NPAIR], [16, 4], [1, 16]]))
                rs128 = sp.tile([128, NPAIR], BF16, tag="rs128")
                with nc.allow_low_precision(reason="softmax rs in bf16"):
                    nc.vector.reciprocal(rs128[:], s128[:])
                nc.sync.dma_start(out=rs2_dr[blk], in_=rs128[:])
                rs_bc = rsJ[blk % 2]
                rsv = rs_bc.rearrange("p (pr two b) -> p pr two b",
                                      two=2, b=B)
                for rp in range(2):
                    src = bass.AP(
                        tensor=rs2_dr.tensor,
                        offset=rs2_dr.offset + blk * 128 * NPAIR
                        + rp * NPAIR * B,
                        ap=[[0, 64], [B, NPAIR], [1, B]])
                    nc.sync.dma_start(
                        out=rsv[rp * 64:(rp + 1) * 64, :, rp, :], in_=src)
                # e_n = e * rs in place (junk quadrants multiplied by 0);
                # the sums matmuls read e_sb before this, so Tile orders it.
                e_n = e_sb
                nc.vector.tensor_tensor(out=e_n[:], in0=e_sb[:],
                                        in1=rs_bc[:],
                                        op=mybir.AluOpType.mult)
                if dbg and blk == 0:
                    nc.sync.dma_start(out=dbg_s.ap(), in_=s_dr[0])
                    nc.sync.dma_start(out=dbg_s128.ap(), in_=s128[:])
                    nc.sync.dma_start(out=dbg_rs128.ap(), in_=rs128[:])
                    nc.sync.dma_start(out=dbg_rsbc.ap(), in_=e_n[:])
                    nc.sync.dma_start(out=dbg_e.ap(), in_=e_sb[:])

                prev = (blk, vb_sb, e_n)

            mix_and_stats(*prev)

        if dbg:
            for ct in range(2):
                nc.sync.dma_start(out=dbg_virt.ap()[ct],
                                  in_=virt[ct][:, 0:2048])

        # =================== STAGE 1.5: stats + collective ===============
        with ExitStack() as s15:
          if do_stats:
              st = s15.enter_context(tc.tile_pool(name="stsb", bufs=1))  # noqa
              # fold acc (z, b) -> per-b raw sums: [1, 2B] = (S1 | S2)
              loc = st.tile([1, 2 * B], F32, tag="loc")
              nc.vector.tensor_reduce(
                  out=loc[:, 0:B],
                  in_=acc1.rearrange("p (z b) -> p b z", b=B),
                  axis=mybir.AxisListType.X, op=mybir.AluOpType.add)
              nc.vector.tensor_reduce(
                  out=loc[:, B:2 * B],
                  in_=acc2.rearrange("p (z b) -> p b z", b=B),
                  axis=mybir.AxisListType.X, op=mybir.AluOpType.add)

              cc_in = dram.tile([1, 2 * B], F32)
              cc_out = dram.tile([1, 2 * B], F32)
              nc.sync.dma_start(out=cc_in[:], in_=loc[:])
              if do_coll:
                  nc.gpsimd.collective_compute(
                      "AllReduce", mybir.AluOpType.add,
                      replica_groups=[list(range(NCORES))],
                      ins=[cc_in.opt()], outs=[cc_out.opt()])
                  bc_src = cc_out
              else:
                  bc_src = cc_in
              stat_bc = st.tile([128, 2 * B], F32, tag="statbc")
              nc.sync.dma_start(
                  out=stat_bc[:],
                  in_=bass.AP(tensor=bc_src.tensor, offset=bc_src.offset,
                              ap=[[0, 128], [1, 2 * B]]))

              # global mean/var/rstd  (raw sums over (C, L) across cores)
              inv = 1.0 / (C * L)
              mean_g = persist.tile([128, B], F32, tag="meang")
              nc.scalar.mul(mean_g[:], stat_bc[:, 0:B], inv)
              var_g = st.tile([128, B], F32, tag="varg")
              nc.scalar.mul(var_g[:], stat_bc[:, B:2 * B], inv)
              msq = st.tile([128, B], F32, tag="msq")
              nc.vector.tensor_tensor(out=msq[:], in0=mean_g[:], in1=mean_g[:],
                                      op=mybir.AluOpType.mult)
              nc.vector.tensor_tensor(out=var_g[:], in0=var_g[:], in1=msq[:],
                                      op=mybir.AluOpType.subtract)
              sd = st.tile([128, B], F32, tag="sd")
              eps_t = st.tile([128, 1], F32, tag="epst")
              nc.vector.memset(eps_t[:], EPS)
              nc.scalar.activation(sd[:], var_g[:], AF.Sqrt, bias=eps_t[:])
              rstd = st.tile([128, B], F32, tag="rstd")
              nc.vector.reciprocal(rstd[:], sd[:])

              # A[ct] = rstd * gamma_c ; Bb[ct] = beta_c - mean * A
              A_sb = [persist.tile([128, B], F32, tag=f"A{ct}", name=f"A{ct}") for ct in range(2)]
              B_sb = [persist.tile([128, B], F32, tag=f"Bb{ct}", name=f"Bb{ct}") for ct in range(2)]
              for ct in range(2):
                  nc.vector.tensor_scalar_mul(A_sb[ct][:], rstd[:], gam_sb[ct][:])
                  tmp = st.tile([128, B], F32, tag="tmpB")
                  nc.vector.tensor_tensor(out=tmp[:], in0=mean_g[:],
                                          in1=A_sb[ct][:],
                                          op=mybir.AluOpType.mult)
                  nc.scalar.activation(B_sb[ct][:], tmp[:], AF.Identity,
                                       bias=bet_sb[ct][:], scale=-1.0)

        # =================== STAGE 2 =====================================
        # out = Wout x + (Wout Wc) relu(A virt + B): the residual add is
        # folded into the PSUM accumulation (wc_sb holds host-fused Wout@Wc).
        with ExitStack() as s2:
          if do_s2:
              p2 = s2.enter_context(tc.tile_pool(name="s2sb", bufs=3))
              po = s2.enter_context(tc.tile_pool(name="po", bufs=8, space="PSUM"))

              virt3s = [virt[ct].rearrange("p (q b) -> p q b", b=B)
                        for ct in range(2)]

              def load_x_and_ox(b):
                  xf_t = [p2.tile([128, LC], BF16, tag=f"xt{ct}", name=f"xt{ct}")
                          for ct in range(2)]
                  for ct in range(2):
                      nc.sync.dma_start(out=xf_t[ct][:], in_=xf.ap()[b, ct])
                  ops = []
                  for ot in range(2):
                      op_ = po.tile([128, LC], F32, tag="wop", name="op_")
                      nc.tensor.matmul(op_[:], wo_sb[0][:, ot * 128:(ot + 1) * 128],
                                       xf_t[0][:], start=True, stop=False)
                      nc.tensor.matmul(op_[:], wo_sb[1][:, ot * 128:(ot + 1) * 128],
                                       xf_t[1][:], start=False, stop=False)
                      ops.append(op_)
                  return ops

              # residual convs for the first few samples run before the
              # collective result lands, hiding AllReduce latency.
              NPRE = 4
              pre = [load_x_and_ox(b) for b in range(NPRE)]

              for b in range(B):
                  rc = [p2.tile([128, LC], BF16, tag=f"rc{ct}", name=f"rc{ct}") for ct in range(2)]
                  # ct0 relu on ScalarE; ct1 affine+relu on DVE
                  nc.scalar.activation(
                      rc[0][:], virt3s[0][:, :, b], AF.Relu,
                      bias=B_sb[0][:, b:b + 1], scale=A_sb[0][:, b:b + 1])
                  nc.vector.tensor_scalar(
                      out=rc[1][:], in0=virt3s[1][:, :, b],
                      scalar1=A_sb[1][:, b:b + 1], scalar2=B_sb[1][:, b:b + 1],
                      op0=mybir.AluOpType.mult, op1=mybir.AluOpType.add)
                  nc.vector.tensor_scalar_max(rc[1][:], rc[1][:], 0.0)
                  ops = pre[b] if b < NPRE else load_x_and_ox(b)
                  for ot in range(2):
                      op_ = ops[ot]
                      nc.tensor.matmul(op_[:], wc_sb[0][:, ot * 128:(ot + 1) * 128],
                                       rc[0][:], start=False, stop=False)
                      nc.tensor.matmul(op_[:], wc_sb[1][:, ot * 128:(ot + 1) * 128],
                                       rc[1][:], start=False, stop=True)
                      o_sb = p2.tile([128, LC], BF16, tag=f"o{ot}")
                      if ot == 0:
                          nc.scalar.copy(o_sb[:], op_[:])
                      else:
                          nc.vector.tensor_copy(o_sb[:], op_[:])
                      nc.sync.dma_start(out=out.ap()[b, ot], in_=o_sb[:])

    nc.compile()
    return nc


def kernel(x, Wq, Wk, Wv, Wc, Wout, gamma, beta):
    x = np.asarray(x)
    Wq, Wk, Wv, Wc, Wout = map(np.asarray, (Wq, Wk, Wv, Wc, Wout))
    gamma, beta = np.asarray(gamma), np.asarray(beta)

    if "nc" not in _CACHE:
        _CACHE["nc"] = build()
    nc = _CACHE["nc"]

    # host weight prep
    M = (Wq.T @ Wk) / np.sqrt(np.float32(C))
    Wf = (Wout @ Wc).astype(np.float32)   # fused: out = Wout x + Wf relu(..)
    wg = np.ascontiguousarray(M.T.reshape(2, 128, C)).astype(BF16NP)
    wv = np.ascontiguousarray(Wv.T.reshape(2, 128, C)).astype(BF16NP)
    wc = np.ascontiguousarray(Wf.T.reshape(2, 128, C)).astype(BF16NP)
    wo = np.ascontiguousarray(Wout.T.reshape(2, 128, C)).astype(BF16NP)
    gb = np.stack([gamma.reshape(2, 128), beta.reshape(2, 128)]).astype(np.float32)
    gb = np.ascontiguousarray(gb)

    in_maps = []
    for m in range(NCORES):
        xs = np.ascontiguousarray(x[:, :, m * LC:(m + 1) * LC], dtype=np.float32)
        xb = np.ascontiguousarray(
            xs.transpose(1, 2, 0).reshape(2, 128, LC, B)).astype(BF16NP)
        xf = np.ascontiguousarray(xs.reshape(B, 2, 128, LC)).astype(BF16NP)
        in_maps.append(dict(xb=xb, xf=xf, wg=wg, wv=wv, wc=wc, wo=wo, gb=gb))

    import os
    trace = bool(int(os.environ.get("KERNEL_TRACE", "0")))
    res = run_bass_kernel_spmd(nc, in_maps, core_ids=list(range(NCORES)),
                               trace=trace)
    _CACHE["last_result"] = res
    shards = [res.results[m]["out"].astype(np.float32).reshape(B, C, LC)
              for m in range(NCORES)]
    return np.concatenate(shards, axis=2)

